# revision 2
# baseline (speedup 1.0000x reference)
"""HDC encoder kernel v4 for Trainium2 (8 NeuronCores, data-parallel over tokens).

Math (per batch row, T tokens, D=1024):
    vecs = token_vectors[tokens]                      # [T, D], entries +-1
    s1[t] = vecs[t]
    s_o[t] = roll(s_{o-1}[t], 1) * vecs[t-(o-1)]      # o = 2..4 (zero-padded in time)
    bundle = (s1+s2+s3+s4)/4
    out = bundle @ W.T + b

v4 structure (per core, 1024 tokens + 4-col front halo):
  - ONE transposing dma_gather pulls bf16 table rows for positions -4..1023
    straight into the d-on-partition layout vbt_a[p, c, col] = vec[d=8p+c]
    (host pre-shuffles table columns; all-zero row at index VOCAB serves
    the front halo; tail positions idx -1 are trimmed by the ucode).
    single_packet=False — the concatenated-packet path overflows the 16-bit
    packet length at this descriptor count.  Extra gather instructions cost
    ~30-50us serialized device time each, so everything runs off this one.
  - Time offsets: V[t'-k] = A@(4-k); odd k reads are 2-byte misaligned and
    run at DVE 1x — cheaper than a second shifted gather.
  - Feature roll c==0 plane: partition rotations of V planes 5/6/7 are
    hoisted to three PE permutation matmuls right after the gather (RA
    buffers); the per-order rotated rows follow from rot(x*y) =
    rot(x)*rot(y), so the DVE phase has NO PE dependency at all.
  - PE then runs the output matmuls as bundle halves complete (tb=0 during
    DVE half 1).  ACT adds bias while casting psum->bf16; DMA per (e, tb).
Host: upcast bf16 -> f32, transpose each core's out^T, assemble [B, T, E].
"""

import numpy as np
import ml_dtypes

import concourse.bass as bass
import concourse.mybir as mybir
import concourse.tile as tile
from concourse import bacc
from concourse.bass import ts, ds
import concourse.bass_utils as bass_utils

VOCAB = 32000
D = 1024
B, T = 4, 2048
NCORES = 8
HALO = 4             # front halo cols (positions t'-4 .. t'-1)
NPOS = 1024 + HALO   # valid gather positions per core (cols 0..1027)
NIDX = 1152          # gather columns (multiple of 128 >= NPOS)
ZROW = VOCAB         # index of the host-appended all-zero table row

_cached = {}


def _build_nc():
    f32 = mybir.dt.float32
    bf16 = mybir.dt.bfloat16
    i16 = mybir.dt.int16
    nc = bacc.Bacc("TRN2", target_bir_lowering=False, debug=False,
                   enable_asserts=False, num_devices=NCORES)

    table = nc.dram_tensor("table", [VOCAB + 1, D], bf16, kind="ExternalInput").ap()
    idxa = nc.dram_tensor("idxa", [128, NIDX // 16], i16, kind="ExternalInput").ap()
    wtc = nc.dram_tensor("wtc", [128, 8 * D], bf16, kind="ExternalInput").ap()
    biasb = nc.dram_tensor("biasb", [128, 8], f32, kind="ExternalInput").ap()
    prot = nc.dram_tensor("prot", [128, 128], bf16, kind="ExternalInput").ap()
    outT = nc.dram_tensor("outT", [D, 1024], bf16, kind="ExternalOutput").ap()

    with tile.TileContext(nc) as tc:
        with tc.tile_pool(name="cst", bufs=1) as cst, \
             tc.tile_pool(name="vbt", bufs=1) as vbtp, \
             tc.tile_pool(name="sng", bufs=1) as sng, \
             tc.tile_pool(name="otile", bufs=4) as otp, \
             tc.tile_pool(name="prm", bufs=3, space="PSUM") as prm, \
             tc.tile_pool(name="psm", bufs=4, space="PSUM") as psm:

            idxa_t = cst.tile([128, NIDX // 16], i16)
            nc.sync.dma_start(out=idxa_t[:], in_=idxa[:])
            biasb_t = cst.tile([128, 8], f32)
            nc.sync.dma_start(out=biasb_t[:], in_=biasb[:])
            wtc_t = cst.tile([128, 8, D], bf16)
            nc.sync.dma_start(out=wtc_t[:], in_=wtc.rearrange("p (c e) -> p c e", c=8))
            prot_t = cst.tile([128, 128], bf16)
            nc.sync.dma_start(out=prot_t[:], in_=prot[:])

            # vbt_a[p, c, col] = vec_at_position(col)[8p+c]
            vbt_a = vbtp.tile([128, 8, NIDX], bf16, tag="vbta")
            # the gather writes cols < roundup(NPOS,16)=1040; zero the rest so
            # the full-width RA rotations below read defined data
            nc.vector.memset(vbt_a[:, :, 1040:NIDX], 0.0)
            nc.gpsimd.dma_gather(
                vbt_a[:], table, idxa_t[:], NIDX, NPOS, D,
                transpose=True, single_packet=False)
            A = vbt_a

            # RA[j] = partition-rotate of V plane (5+j) over all columns.
            # rot(x)[p] = x[(p-1) % 128] via permutation matmul.
            ra = vbtp.tile([128, 3, NIDX], bf16, tag="ra")
            for j, pl in enumerate((5, 6, 7)):
                for q in range(3):
                    ps = prm.tile([128, 384], f32, tag="rot", name=f"ra{pl}_{q}")
                    nc.tensor.matmul(out=ps[:], lhsT=prot_t[:],
                                     rhs=A[:, pl, ts(q, 384)],
                                     start=True, stop=True)
                    nc.scalar.copy(out=ra[:, j, ts(q, 384)], in_=ps[:])
            RA5, RA6, RA7 = ra[:, 0, :], ra[:, 1, :], ra[:, 2, :]

            s2 = sng.tile([128, 8, 1024], bf16, tag="s2")
            s3 = sng.tile([128, 8, 1024], bf16, tag="s3")
            s4 = sng.tile([128, 8, 1024], bf16, tag="s4")
            bnd = sng.tile([128, 8, 1024], bf16, tag="bnd")

            CH = 256  # t' chunk for DVE pipelining
            mul = nc.vector.tensor_mul
            add = nc.vector.tensor_add
            for h in range(2):            # 512-wide halves — pure DVE
                hb = 512 * h
                H = ds(hb, 512)

                for b0 in (hb, hb + 256):
                    C = ds(b0, CH)
                    # s2 = roll(V[t'],1) * V[t'-1]   (V[t']=A@+4, V[t'-1]=A@+3)
                    mul(out=s2[:, 1:8, C], in0=A[:, 0:7, ds(b0 + 4, CH)],
                        in1=A[:, 1:8, ds(b0 + 3, CH)])
                mul(out=s2[:, 0, H], in0=RA7[:, ds(hb + 4, 512)],
                    in1=A[:, 0, ds(hb + 3, 512)])

                for b0 in (hb, hb + 256):
                    C = ds(b0, CH)
                    # s3 = roll(s2,1) * V[t'-2]      (V[t'-2]=A@+2)
                    mul(out=s3[:, 1:8, C], in0=s2[:, 0:7, C],
                        in1=A[:, 1:8, ds(b0 + 2, CH)])
                # rot(s2[,7]) = rot(A6@+4 * A7@+3) = RA6@+4 * RA7@+3
                t_rs2 = otp.tile([128, 512], bf16, tag="trs2", name=f"trs2{h}")
                mul(out=t_rs2[:], in0=RA6[:, ds(hb + 4, 512)],
                    in1=RA7[:, ds(hb + 3, 512)])
                mul(out=s3[:, 0, H], in0=t_rs2[:], in1=A[:, 0, ds(hb + 2, 512)])

                for b0 in (hb, hb + 256):
                    C = ds(b0, CH)
                    # s4 = roll(s3,1) * V[t'-3]      (V[t'-3]=A@+1)
                    mul(out=s4[:, 1:8, C], in0=s3[:, 0:7, C],
                        in1=A[:, 1:8, ds(b0 + 1, CH)])
                # rot(s3[,7]) = RA5@+4 * RA6@+3 * RA7@+2
                t1 = otp.tile([128, 512], bf16, tag="t1", name=f"t1{h}")
                mul(out=t1[:], in0=RA5[:, ds(hb + 4, 512)],
                    in1=RA6[:, ds(hb + 3, 512)])
                t_rs3 = otp.tile([128, 512], bf16, tag="trs3", name=f"trs3{h}")
                mul(out=t_rs3[:], in0=t1[:], in1=RA7[:, ds(hb + 2, 512)])
                mul(out=s4[:, 0, H], in0=t_rs3[:], in1=A[:, 0, ds(hb + 1, 512)])

                # bundle = V[t'] + s2 + s3 + s4
                add(out=bnd[:, :, H], in0=A[:, 0:8, ds(hb + 4, 512)],
                    in1=s2[:, :, H])
                add(out=bnd[:, :, H], in0=bnd[:, :, H], in1=s3[:, :, H])
                add(out=bnd[:, :, H], in0=bnd[:, :, H], in1=s4[:, :, H])

            # out^T[e, t'] = sum_c wtc[:, c, e_blk].T @ bundle[:, c, t_blk]
            # tb outer: the tb=0 block runs on PE while DVE computes half 1.
            for tb in range(2):
                for e in range(8):
                    pm = psm.tile([128, 512], f32, tag="mps", name=f"pm{tb}_{e}")
                    for c in range(8):
                        nc.tensor.matmul(
                            out=pm[:],
                            lhsT=wtc_t[:, c, ts(e, 128)],
                            rhs=bnd[:, c, ts(tb, 512)],
                            start=(c == 0),
                            stop=(c == 7),
                        )
                    ot = otp.tile([128, 512], bf16, tag="ot")
                    nc.scalar.add(out=ot[:], in_=pm[:], add=biasb_t[:, e:e + 1])
                    nc.sync.dma_start(out=outT[ts(e, 128), ts(tb, 512)], in_=ot[:])

    nc.compile()
    return nc


def _prep_inputs(tokens, token_vectors, W, b):
    tokens = np.asarray(tokens)
    W = np.asarray(W, dtype=np.float32)
    b = np.asarray(b, dtype=np.float32)

    # table: bf16, column-shuffled so the gather's d=128q+p transpose lands
    # as d=8p+c, plus an all-zero halo row at index ZROW.
    tv = np.asarray(token_vectors, dtype=np.float32)
    shuf = tv.reshape(VOCAB, 128, 8).transpose(0, 2, 1).reshape(VOCAB, D)
    table = np.zeros((VOCAB + 1, D), dtype=ml_dtypes.bfloat16)
    table[:VOCAB] = shuf.astype(ml_dtypes.bfloat16)

    wt = np.ascontiguousarray((W / 4.0).T)          # [d, e]
    wtc = wt.reshape(128, 8, D).reshape(128, 8 * D)  # d = 8p + c -> [p, (c e)]
    wtc = wtc.astype(ml_dtypes.bfloat16)
    biasb = np.ascontiguousarray(b.reshape(8, 128).T)  # [p, e_blk]
    # cyclic partition rotate: out[m] = in[(m-1) % 128]
    prot = np.zeros((128, 128), dtype=ml_dtypes.bfloat16)
    prot[np.arange(128), (np.arange(128) + 1) % 128] = 1

    def wrap_idx(vals):
        w = np.asarray(vals, dtype=np.int16).reshape(NIDX // 16, 16).T  # [16, S]
        return np.ascontiguousarray(np.tile(w, (8, 1)))

    in_maps = []
    for k in range(NCORES):
        rb, half = k // 2, k % 2
        a = half * 1024
        pos = np.arange(NIDX)
        gt = a - HALO + pos
        ia = np.full(NIDX, -1, dtype=np.int64)
        valid = pos < NPOS
        ia[valid] = np.where(gt[valid] >= 0, tokens[rb, np.maximum(gt[valid], 0)], ZROW)
        in_maps.append({
            "table": table,
            "idxa": wrap_idx(ia),
            "wtc": wtc,
            "biasb": biasb,
            "prot": prot,
        })
    return in_maps


def kernel(tokens, token_vectors, W, b, trace=False):
    if "nc" not in _cached:
        _cached["nc"] = _build_nc()
    nc = _cached["nc"]
    in_maps = _prep_inputs(tokens, token_vectors, W, b)
    res = bass_utils.run_bass_kernel_spmd(
        nc, in_maps, core_ids=list(range(NCORES)), trace=trace,
    )
    _cached["last_result"] = res
    out = np.zeros((B, T, D), dtype=np.float32)
    for k in range(NCORES):
        rb, half = k // 2, k % 2
        out[rb, half * 1024:(half + 1) * 1024, :] = \
            res.results[k]["outT"].astype(np.float32).T
    return out
